# revision 25
# baseline (speedup 1.0000x reference)
"""Trainium2 Bass kernel for nn_AsynBottleneck (sparse active-site bottleneck block).

Self-contained: hardcodes all shapes. Strategy:
  - 8-way data-parallel over active sites (contiguous, coords sorted), halo overlap.
  - conv1 (1x1): TensorE, channels-on-partitions, bf16; psum M=128 with duplicated
    output channels so the upper half can be written column-shifted (dx=+1 tap).
  - sparse 3x3 conv: per-core HBM slab indexed by dense grid position; slab row r
    (256B, bf16) = [out1[pos r] | out1[pos r + 2*W')]  (2W' row-pair K-stacking).
    Built by two dma_scatter_add calls into runtime-pre-zeroed ExternalOutput DRAM.
    The 6 dy!=0 taps come from two dma_gather(transpose) window descriptors per
    site (512B + 256B elems); the 3 dy=0 taps come from shifted compact reads of
    out1T in SBUF with a host-precomputed edge-validity mask.
  - conv3 (1x1): stationary w3 (+bias row), channel-major psum; residual added from
    the already-resident transposed feats; bf16 channel-major output, transposed
    back to [N, 256] f32 on host.
"""
import os
import sys

for _p in ("/opt/trn_rl_repo", "/root/.axon_site/_ro/trn_rl_repo"):
    if os.path.isdir(_p) and _p not in sys.path:
        sys.path.insert(0, _p)

import numpy as np
import ml_dtypes

from concourse import bass, mybir, bacc
import concourse.tile as tile
from concourse.bass import AP
from concourse.bass_utils import run_bass_kernel_spmd
from concourse.masks import make_identity

BF16 = ml_dtypes.bfloat16

# problem constants
HG, WG = 768, 768
N = 200000
CIN, CB = 256, 64
NCORES = 8

WP = WG + 2            # padded grid row stride (770)
DELTA = 2 * WP         # slab stacking shift (1540)

# per-core tiling
S_CORE = N // NCORES   # 25000
CH = 3584              # output sites per chunk
NCHUNK = 7
S_PAD = CH * NCHUNK    # 25088
HALO = 512
WIN = CH + 2 * HALO    # 4608
SUBT = WIN // 512      # 9 conv1 subtiles
TPC = CH // 512        # 7 output tiles per chunk
WEXT = S_PAD + 2 * HALO  # 26112 window sites per core
SSPAN = 12288          # slab rows per chunk
DUMP = SSPAN - 1

LAST_EXEC_NS = None
_GRAPH_CACHE = {}


def _install_ntff_hook():
    """Provide antenv.axon_hooks (missing in this image) so trace=True works."""
    import contextlib
    import ctypes
    import types
    try:
        from antenv.axon_hooks import get_axon_ntff_profile_hook  # noqa
        return
    except ImportError:
        pass
    so_path = "/opt/axon/libaxon_pjrt.so"
    if not os.path.exists(so_path):
        return
    lib = ctypes.CDLL(so_path)
    if not hasattr(lib, "axon_start_nrt_profile"):
        return
    lib.axon_start_nrt_profile.argtypes = [ctypes.POINTER(ctypes.c_int64), ctypes.c_size_t]
    lib.axon_start_nrt_profile.restype = ctypes.c_int64
    lib.axon_stop_nrt_profile.argtypes = [ctypes.c_char_p]
    lib.axon_stop_nrt_profile.restype = ctypes.c_int64

    @contextlib.contextmanager
    def _hook(output_dir, device_ids):
        import jax
        jax.devices()
        if device_ids:
            ids = (ctypes.c_int64 * len(device_ids))(*device_ids)
            rc = lib.axon_start_nrt_profile(ids, len(device_ids))
        else:
            rc = lib.axon_start_nrt_profile(None, 0)
        if rc != 0:
            raise RuntimeError(f"axon_start_nrt_profile rc={rc}")
        try:
            yield
        finally:
            lib.axon_stop_nrt_profile(str(output_dir).encode())

    holder = [_hook]
    mod = types.ModuleType("antenv.axon_hooks")
    mod.get_axon_ntff_profile_hook = lambda: holder[0]
    mod.set_axon_ntff_profile_hook = lambda h: holder.__setitem__(0, h)
    sys.modules["antenv.axon_hooks"] = mod
    import antenv
    antenv.axon_hooks = mod


def _wrap16(vals):
    """[n] -> [128, n//16] int16, wrapped in 16 partitions, replicated x8."""
    n = vals.shape[0]
    base = vals.reshape(n // 16, 16).T.astype(np.int16)   # [16, n//16]
    return np.tile(base, (8, 1))


def _build_graph():
    if "nc" in _GRAPH_CACHE:
        return _GRAPH_CACHE["nc"]
    nc = bacc.Bacc("TRN2", target_bir_lowering=False, debug=False, num_devices=NCORES,
                   num_swdge_queues=4, dynamic_dma_scratch_size=32768)
    dt = mybir.dt

    fA = nc.declare_dram_parameter("fA", [128, WEXT], dt.bfloat16, isOutput=False)
    fB = nc.declare_dram_parameter("fB", [128, WEXT], dt.bfloat16, isOutput=False)
    w1 = nc.declare_dram_parameter("w1", [128, 2, 128], dt.bfloat16, isOutput=False)
    b1 = nc.declare_dram_parameter("b1", [128, 1], dt.float32, isOutput=False)
    w2 = nc.declare_dram_parameter("w2", [128, 5, 64], dt.bfloat16, isOutput=False)
    b2 = nc.declare_dram_parameter("b2", [64, 1], dt.float32, isOutput=False)
    w3 = nc.declare_dram_parameter("w3", [65, 2, 128], dt.bfloat16, isOutput=False)
    scat = nc.declare_dram_parameter("scat", [NCHUNK, 128, 2, WIN // 16], dt.int16, isOutput=False)
    gath = nc.declare_dram_parameter("gath", [NCHUNK, 128, CH // 16], dt.int16, isOutput=False)
    emsk = nc.declare_dram_parameter("emsk", [NCHUNK, 64, WIN], dt.bfloat16, isOutput=False)
    outT = nc.declare_dram_parameter("outT", [2, 128, S_PAD], dt.bfloat16, isOutput=True)
    slabs = [
        nc.declare_dram_parameter(f"slab{k}", [SSPAN, 128], dt.bfloat16, isOutput=True)
        for k in range(NCHUNK)
    ]

    nchunk_run = int(os.environ.get("BASSNN_NCHUNK", NCHUNK))
    with tile.TileContext(nc) as tc:
        with tc.tile_pool(name="consts", bufs=1) as cpool, \
             tc.tile_pool(name="f", bufs=2) as fpool, \
             tc.tile_pool(name="o1", bufs=2) as o1pool, \
             tc.tile_pool(name="g", bufs=2) as gpool, \
             tc.tile_pool(name="o2p", bufs=1) as o2pool, \
             tc.tile_pool(name="misc", bufs=2) as mpool, \
             tc.tile_pool(name="gi", bufs=2) as gipool, \
             tc.tile_pool(name="onat", bufs=3) as onatpool, \
             tc.tile_pool(name="ostp", bufs=2) as ostpool, \
             tc.tile_pool(name="ps", bufs=2, space="PSUM") as pspool, \
             tc.tile_pool(name="ps1", bufs=2, space="PSUM") as ps1pool:

            # ---- constants ----
            w1t = cpool.tile([128, 2, 128], dt.bfloat16)
            nc.sync.dma_start(w1t[:], w1[:])
            b1t = cpool.tile([128, 1], dt.float32)
            nc.sync.dma_start(b1t[:], b1[:])
            w2t = cpool.tile([128, 5, 64], dt.bfloat16)
            nc.sync.dma_start(w2t[:], w2[:])
            b2t = cpool.tile([64, 1], dt.float32)
            nc.sync.dma_start(b2t[:], b2[:])
            w3t = cpool.tile([65, 2, 128], dt.bfloat16)
            nc.sync.dma_start(w3t[:], w3[:])
            ident = cpool.tile([64, 64], dt.bfloat16)
            make_identity(nc, ident[:])

            front = {}

            def stage_front(k):
                """load + conv1 + transposes + scatters for chunk k."""
                r0 = k * CH  # fA row base for this chunk's window

                # ---- index / mask loads (small, critical-path: first) ----
                sidx = mpool.tile([128, 2, WIN // 16], dt.int16, tag="sidx")
                nc.sync.dma_start(sidx[:], scat[k])
                gidx = gipool.tile([128, CH // 16], dt.int16, tag="gidx")
                nc.sync.dma_start(gidx[:], gath[k])
                e128 = mpool.tile([128, WIN], dt.bfloat16, tag="e128")
                nc.sync.dma_start(e128[0:64, :], emsk[k])
                nc.sync.dma_start(e128[64:128, :], emsk[k])

                # ---- load feats window (host-pre-transposed, channel-major) ----
                fT0 = fpool.tile([128, WIN], dt.bfloat16, tag="fT0")
                fT1 = fpool.tile([128, WIN], dt.bfloat16, tag="fT1")
                nc.sync.dma_start(fT0[:], fA[:, r0:r0 + WIN])
                nc.sync.dma_start(fT1[:], fB[:, r0:r0 + WIN])

                # ---- conv1 ----
                # O1: rows 0-63 = out1T (plain), rows 64-127 = out1T shifted left 1
                # (later masked in place -> A). M2 = out1T * e.
                O1 = o1pool.tile([128, WIN], dt.bfloat16, tag="O1")
                for s in range(SUBT):
                    ps1 = ps1pool.tile([128, 512], dt.float32, tag="ps1", space="PSUM")
                    nc.tensor.matmul(out=ps1[:], lhsT=w1t[:, 0, :],
                                     rhs=fT0[:, s * 512:(s + 1) * 512],
                                     start=True, stop=False)
                    nc.tensor.matmul(out=ps1[:], lhsT=w1t[:, 1, :],
                                     rhs=fT1[:, s * 512:(s + 1) * 512],
                                     start=False, stop=True)
                    nc.scalar.activation(out=O1[0:64, s * 512:(s + 1) * 512],
                                         in_=ps1[0:64, :],
                                         func=mybir.ActivationFunctionType.Relu,
                                         bias=b1t[0:64, :], scale=1.0)
                    if s == 0:
                        nc.scalar.activation(out=O1[64:128, 0:511],
                                             in_=ps1[64:128, 1:512],
                                             func=mybir.ActivationFunctionType.Relu,
                                             bias=b1t[64:128, :], scale=1.0)
                    else:
                        nc.scalar.activation(out=O1[64:128, s * 512 - 1:s * 512 + 511],
                                             in_=ps1[64:128, :],
                                             func=mybir.ActivationFunctionType.Relu,
                                             bias=b1t[64:128, :], scale=1.0)
                # A-mask (in place on upper half) and M2
                M2 = o1pool.tile([64, WIN], dt.bfloat16, tag="M2")
                nc.vector.tensor_mul(out=O1[64:128, 0:WIN - 1],
                                     in0=O1[64:128, 0:WIN - 1],
                                     in1=e128[64:128, 0:WIN - 1])
                nc.vector.tensor_mul(out=M2[:], in0=O1[0:64, :], in1=e128[0:64, :])

                # ---- transpose out1T -> sites-on-partitions (for the scatter) ----
                o1nat = onatpool.tile([128, WIN // 128, 64], dt.bfloat16, tag="o1nat")
                for g in range(SUBT):  # 9 groups of 4 transposes
                    pst = ps1pool.tile([128, 4, 64], dt.bfloat16, tag="pst", space="PSUM")
                    for u in range(4):
                        w = g * 4 + u
                        nc.tensor.transpose(out=pst[:, u, :],
                                            in_=O1[0:64, w * 128:(w + 1) * 128],
                                            identity=ident[:])
                    nc.vector.tensor_copy(out=o1nat[:, g * 4:(g + 1) * 4, :], in_=pst[:])

                # ---- scatter into the slab ----
                slab = slabs[k]
                nc.gpsimd.dma_scatter_add(
                    out_ap=slab[:, 0:64], in_ap=o1nat[:], idxs_ap=sidx[:, 0, :],
                    num_idxs=WIN, num_idxs_reg=WIN, elem_size=64, elem_step=128,
                    single_packet=False, queue_num=(3 * k) % 4)
                nc.gpsimd.dma_scatter_add(
                    out_ap=slab[:, 64:128], in_ap=o1nat[:], idxs_ap=sidx[:, 1, :],
                    num_idxs=WIN, num_idxs_reg=WIN, elem_size=64, elem_step=128,
                    single_packet=False, queue_num=(3 * k + 1) % 4)
                front[k] = (fT0, fT1, gidx, O1, M2)

            def stage_back(k):
                """gather + conv2 + conv3 + store for chunk k."""
                fT0, fT1, gidx, O1, M2 = front.pop(k)
                slab = slabs[k]

                # ---- gather dy!=0 tap windows (3 rows -> 6 taps) ----
                gA = gpool.tile([128, 3, CH], dt.bfloat16, tag="gA")
                winA = AP(slab.ap().tensor, 0, [[128, SSPAN - 2], [1, 384]])
                nc.gpsimd.dma_gather(
                    out_ap=gA[:], in_ap=winA, idxs_ap=gidx[:],
                    num_idxs=CH, num_idxs_reg=CH,
                    elem_size=384, elem_step=128, transpose=True,
                    single_packet=False, queue_num=(3 * k + 2) % 4)

                # ---- conv2 + bn2 + relu ----
                o2 = o2pool.tile([65, CH], dt.bfloat16, tag="o2")
                nc.vector.memset(o2[64:65, :], 1.0)
                for t in range(TPC):
                    sl = slice(t * 512, (t + 1) * 512)
                    ps2 = pspool.tile([64, 512], dt.float32, tag="ps2", space="PSUM")
                    nc.tensor.matmul(out=ps2[:], lhsT=w2t[:, 0, :], rhs=gA[:, 0, sl],
                                     start=True, stop=False)
                    nc.tensor.matmul(out=ps2[:], lhsT=w2t[:, 1, :], rhs=gA[:, 1, sl],
                                     start=False, stop=False)
                    nc.tensor.matmul(out=ps2[:], lhsT=w2t[:, 2, :], rhs=gA[:, 2, sl],
                                     start=False, stop=False)
                    c0 = HALO + t * 512
                    nc.tensor.matmul(out=ps2[:], lhsT=w2t[:, 3, :],
                                     rhs=O1[:, c0:c0 + 512],
                                     start=False, stop=False)
                    nc.tensor.matmul(out=ps2[:], lhsT=w2t[0:64, 4, :],
                                     rhs=M2[:, c0 - 1:c0 + 511],
                                     start=False, stop=True)
                    nc.scalar.activation(out=o2[0:64, sl], in_=ps2[:],
                                         func=mybir.ActivationFunctionType.Relu,
                                         bias=b2t[:], scale=1.0)

                # ---- conv3 + residual + relu (channel-major) ----
                ost = ostpool.tile([128, 2, CH], dt.bfloat16, tag="ost")
                for h in range(2):
                    fTh = fT0 if h == 0 else fT1
                    for t in range(TPC):
                        sl = slice(t * 512, (t + 1) * 512)
                        ps3 = pspool.tile([128, 512], dt.float32, tag="ps3", space="PSUM")
                        nc.tensor.matmul(out=ps3[:], lhsT=w3t[:, h, :], rhs=o2[:, sl],
                                         start=True, stop=True)
                        c0 = HALO + t * 512
                        nc.vector.tensor_add(out=ost[:, h, sl], in0=ps3[:],
                                             in1=fTh[:, c0:c0 + 512])
                        nc.scalar.activation(out=ost[:, h, sl], in_=ost[:, h, sl],
                                             func=mybir.ActivationFunctionType.Relu,
                                             bias=0.0, scale=1.0)
                    nc.sync.dma_start(outT[h, :, k * CH:(k + 1) * CH], ost[:, h, :])

            # software pipeline: front stage runs one chunk ahead
            stage_front(0)
            for k in range(nchunk_run):
                if k + 1 < nchunk_run:
                    stage_front(k + 1)
                stage_back(k)

    nc.compile()
    _GRAPH_CACHE["nc"] = nc
    return nc


def _prep_core(c, coords, pos, xg, feats_bf, e_glob):
    """Build per-core input arrays."""
    g0 = c * S_CORE

    # window of WEXT sites, local l in [-HALO, S_PAD+HALO)
    gl_lo = g0 - HALO
    fwin = np.zeros((WEXT, CIN), dtype=BF16)
    a = max(0, gl_lo)
    b = min(N, gl_lo + WEXT)
    if b > a:
        fwin[a - gl_lo:b - gl_lo] = feats_bf[a:b]

    # window-global site id and validity
    gl = gl_lo + np.arange(WEXT)
    real = (gl >= 0) & (gl < N)
    posw = np.where(real, pos[np.clip(gl, 0, N - 1)], -10**9)

    scat_arr = np.zeros((NCHUNK, 128, 2, WIN // 16), np.int16)
    gath_arr = np.zeros((NCHUNK, 128, CH // 16), np.int16)
    emsk_arr = np.zeros((NCHUNK, 64, WIN), dtype=BF16)

    for k in range(NCHUNK):
        # window cols j=0..WIN-1 <-> window-extended index w = k*CH + j
        w0 = k * CH
        pw = posw[w0:w0 + WIN]
        rw = real[w0:w0 + WIN]
        # output sites: cols [HALO, HALO+CH)
        pout = pw[HALO:HALO + CH]
        rout = rw[HALO:HALO + CH]
        if rout.any():
            rlo = int(pout[rout].min()) - WP - 1 - 4
        else:
            rlo = 0
        # gather indices: rows ra, ra+1, ra+2 = positions p-WP-1, p-WP, p-WP+1
        ra = np.where(rout, pout - WP - 1 - rlo, 0)
        assert ra.min() >= 0
        assert ra.max() <= SSPAN - 3, ra.max()
        gath_arr[k, :, :] = _wrap16(ra)
        # scatter indices (lower: row=p-rlo; upper: row=p-DELTA-rlo)
        r_low = pw - rlo
        r_up = pw - DELTA - rlo
        s_low = np.where(rw & (r_low >= 0) & (r_low < DUMP), r_low, DUMP)
        s_up = np.where(rw & (r_up >= 0) & (r_up < DUMP), r_up, DUMP)
        scat_arr[k, :, 0, :] = _wrap16(s_low)
        scat_arr[k, :, 1, :] = _wrap16(s_up)
        # edge mask e[j]: edge between window sites (w0+j, w0+j+1)
        glw = gl[w0:w0 + WIN]
        okl = rw & (glw < N - 1)
        gsafe = np.clip(glw, 0, N - 2)
        e = okl & e_glob[gsafe]
        emsk_arr[k] = np.broadcast_to(e.astype(BF16)[None, :], (64, WIN))

        # completeness check: every gathered row's content must have been scattered
        if rout.any():
            # rows gathered (3-row window)
            need = np.unique(np.concatenate([ra[rout], ra[rout] + 1, ra[rout] + 2]))
            # lower content at row r: site at position r+rlo ; upper: r+rlo+DELTA
            for off, sidx_arr in ((0, s_low), (DELTA, s_up)):
                p_need = need + rlo + off
                # does a real site exist at these positions?
                exist = np.isin(p_need, pw[rw])
                if exist.any():
                    # those sites must scatter to exactly these rows
                    m = np.isin(pw, p_need[exist]) & rw
                    tgt = (pw - rlo) if off == 0 else (pw - DELTA - rlo)
                    ok = (sidx_arr[m] == tgt[m]) & (sidx_arr[m] != DUMP)
                    assert ok.all(), f"core {c} chunk {k} off {off}: missing scatters"

    return dict(fA=np.ascontiguousarray(fwin[:, 0:128].T),
                fB=np.ascontiguousarray(fwin[:, 128:256].T),
                scat=scat_arr, gath=gath_arr, emsk=emsk_arr)


def kernel(feats, coords, w1, w2, w3, s1, b1, s2, b2, s3, b3):
    global LAST_EXEC_NS
    feats = np.asarray(feats, dtype=np.float32)
    coords = np.asarray(coords).astype(np.int64)
    w1 = np.asarray(w1, np.float32); w2 = np.asarray(w2, np.float32)
    w3 = np.asarray(w3, np.float32)
    s1 = np.asarray(s1, np.float32); b1 = np.asarray(b1, np.float32)
    s2 = np.asarray(s2, np.float32); b2 = np.asarray(b2, np.float32)
    s3 = np.asarray(s3, np.float32); b3 = np.asarray(b3, np.float32)

    yg = coords // WG
    xg = coords % WG
    pos = (yg + 1) * WP + (xg + 1)          # padded dense position
    e_glob = np.zeros(N - 1, dtype=bool)     # edge (j, j+1) valid
    e_glob = (coords[1:] == coords[:-1] + 1) & (xg[:-1] < WG - 1)

    feats_bf = feats.astype(BF16)

    # folded weights
    w1f = (w1 * s1[None, :]).astype(np.float32)      # [256, 64]
    w2f = (w2 * s2[None, None, :]).astype(np.float32)  # [9, 64, 64]
    w3f = (w3 * s3[None, :]).astype(np.float32)      # [64, 256]

    w1dup = np.concatenate([w1f, w1f], axis=1)       # [256, 128]
    w1_arr = np.stack([w1dup[0:128], w1dup[128:256]], axis=1).astype(BF16)  # [128,2,128]
    b1_arr = np.tile(b1, 2).reshape(128, 1).astype(np.float32)

    z64 = np.zeros((64, 64), np.float32)
    w2stacks = [
        np.concatenate([w2f[0], w2f[6]], axis=0),   # gA slot0: (-1,-1)+(+1,-1)
        np.concatenate([w2f[1], w2f[7]], axis=0),   # gA slot1: (-1,0)+(+1,0)
        np.concatenate([w2f[2], w2f[8]], axis=0),   # gB: (-1,+1)+(+1,+1)
        np.concatenate([w2f[4], w2f[5]], axis=0),   # O1: center + (0,+1)
        np.concatenate([w2f[3], z64], axis=0),      # M2: (0,-1)
    ]
    w2_arr = np.stack(w2stacks, axis=1).astype(BF16)  # [128, 5, 64]
    b2_arr = b2.reshape(64, 1).astype(np.float32)

    w3b = np.concatenate([w3f, b3[None, :]], axis=0)  # [65, 256]
    w3_arr = np.stack([w3b[:, 0:128], w3b[:, 128:256]], axis=1).astype(BF16)  # [65,2,128]

    nc = _build_graph()
    in_maps = []
    for c in range(NCORES):
        m = _prep_core(c, coords, pos, xg, feats_bf, e_glob)
        m.update(w1=w1_arr, b1=b1_arr, w2=w2_arr, b2=b2_arr, w3=w3_arr)
        in_maps.append(m)

    trace = os.environ.get("BASSNN_TRACE", "0") == "1"
    if trace:
        _install_ntff_hook()
    res = run_bass_kernel_spmd(nc, in_maps, core_ids=list(range(NCORES)), trace=trace)
    LAST_EXEC_NS = res.exec_time_ns

    out = np.empty((N, CIN), np.float32)
    for c in range(NCORES):
        oT = res.results[c]["outT"]  # [2, 128, S_PAD] bf16
        oc = np.concatenate([oT[0], oT[1]], axis=0).T  # [S_PAD, 256]
        nreal = min(S_CORE, N - c * S_CORE)
        out[c * S_CORE:c * S_CORE + nreal] = oc[:nreal].astype(np.float32)
    return out

